# revision 28
# baseline (speedup 1.0000x reference)
"""Trainium2 Bass kernel for nn_C_BatchNorm (complex batch-norm, training mode).

Problem: z [B=32, C=128, H=64, W=64, 2] fp32.  Per position n=(c,h,w):
  2x2 covariance over batch, closed-form inverse sqrt, whiten, gamma/beta.

Sharding: C split across 8 cores (16 channels each).  Per core the shard is
[32, 131072] fp32 (16 MiB in / 16 MiB out), processed as 32 tiles of
[128 partitions = 4 position-groups x 32 batch, 1024 = 512 positions x 2
interleaved components].

v2: all PE matmuls run in bf16 (1 cyc/row, cheap weight loads; coefficient /
moment quantization ~1e-3 rel, well inside the 2e-2 gate).

  Phase A (per tile): DMA 2-tile batches; ACT casts z->bf16 (resident zb);
    DVE squares zb (bf16 TT, 2x mode); GPSIMD strided zb0*zb1; 5 bf16
    matmuls with per-tile selector weights accumulate raw moments
    S/Q/X for block g directly into PSUM partition strip 32g via
    tile_position=(0, 32g) -- no staging copies.
  Phase B (once): closed-form 2x2 inverse-sqrt + gamma fold, reading
    moments straight from PSUM; emits bf16 coefficient planes
    Pap=(A00,A10), Qap=(A01,A11), Rap=(R0,R1) where out = A.z + R.
  Phase C (per tile): K=32 indicator bf16 matmuls broadcast the tile's 4
    coefficient rows to all 128 partitions in PSUM; DVE multiplies with
    stride-0 "dup" views of zb; GPSIMD adds the pair; +R split between
    DVE (PSUM read) and GPSIMD (via ACT-copied R) to balance engines.
"""

import numpy as np

import concourse.bass as bass
import concourse.bacc as bacc
import concourse.tile as tile
from concourse import mybir
from concourse.bass_utils import run_bass_kernel_spmd

f32 = mybir.dt.float32
f32r = mybir.dt.float32r
bf16 = mybir.dt.bfloat16
AF = mybir.ActivationFunctionType
OP = mybir.AluOpType

# ---- problem geometry (hardcoded) ----
B, C, H, W = 32, 128, 64, 64
NCORES = 8
C_PER = C // NCORES                  # 16 channels per core
NPOS = C_PER * H * W                 # 65536 positions per core
M = NPOS * 2                         # 131072 fp32 per batch row per core
NT = 32                              # tiles per core
FP = 512                             # positions per group per tile
COLS = 2 * FP                        # 1024 fp32 per partition per tile
J = 4                                # position groups per tile (32 batch each)
NB = 512                             # matmul free-dim chunk (one PSUM bank)


def _host_constants():
    import ml_dtypes
    # Data partition p = 4*b + j (j = position group = p%4, b = batch = p//4)
    # -- j innermost so the DMA access pattern's outer dim is 32 (batch) and
    # the HWDGE spreads each transfer over all 16 SDMA engines.
    # selector for phase-A stats packing: variant i (tile t=8g+i):
    # sel[p, 32i + w] = 1 iff w == 4i + p%4; stats land on PSUM strip rows
    # 32g + 4i + j via tile_position=(0, 32g).
    sel8 = np.zeros((128, 8, 32), dtype=np.float32)
    for i in range(8):
        for p in range(128):
            sel8[p, i, 4 * i + p % 4] = 1.0
    sel8 = sel8.reshape(128, 8 * 32).astype(ml_dtypes.bfloat16)
    # indicator for phase-C broadcast: 8 variants [32, 128] tiled to 128
    # rows: ind[r, 128i + q] = 1 iff (r % 32) == 4*i + q%4
    ind = np.zeros((128, 8, 128), dtype=np.float32)
    for i in range(8):
        for r in range(128):
            for q in range(128):
                if r % 32 == 4 * i + q % 4:
                    ind[r, i, q] = 1.0
    ind = ind.reshape(128, 8 * 128).astype(ml_dtypes.bfloat16)
    id128 = np.eye(128, dtype=np.float32).astype(ml_dtypes.bfloat16)
    return sel8, ind, id128


def build_module(reps=1):
    nc = bacc.Bacc("TRN2", target_bir_lowering=False, debug=False,
                   detect_race_conditions=False)
    z_d = nc.dram_tensor("z", [B, M], f32, kind="ExternalInput").ap()
    gamma_d = nc.dram_tensor("gamma", [2, 2], f32, kind="ExternalInput").ap()
    beta_d = nc.dram_tensor("beta", [2], f32, kind="ExternalInput").ap()
    sel8_d = nc.dram_tensor("sel8", [128, 8 * 32], bf16, kind="ExternalInput").ap()
    ind_d = nc.dram_tensor("ind", [128, 8 * 128], bf16, kind="ExternalInput").ap()
    id_d = nc.dram_tensor("id128", [128, 128], bf16, kind="ExternalInput").ap()
    out_d = nc.dram_tensor("out", [B, M], f32, kind="ExternalOutput").ap()

    # DRAM views ordered [tile, batch, group, col] (partition p = 4*b + j):
    # outer dim 32 so each DMA fans out over all 16 SDMA engines.
    z_r = z_d.rearrange("b (t j f) -> t b j f", t=NT, j=J, f=COLS)
    out_r = out_d.rearrange("b (t j f) -> t b j f", t=NT, j=J, f=COLS)

    irB = 1.0 / np.sqrt(np.float32(B))      # 1/sqrt(B)

    with tile.TileContext(nc) as tc:
        with (
            tc.tile_pool(name="consts", bufs=1) as consts,
            tc.tile_pool(name="zres", bufs=1) as zres,
            tc.tile_pool(name="ph2", bufs=1) as ph2,
            tc.tile_pool(name="stag", bufs=4) as stagp,
            tc.tile_pool(name="work", bufs=3) as work,
        ):
            # ---------- constants ----------
            sel_sb = consts.tile([128, 8 * 32], bf16)
            nc.sync.dma_start(out=sel_sb[:], in_=sel8_d)
            ind_sb = consts.tile([128, 8 * 128], bf16)
            nc.sync.dma_start(out=ind_sb[:], in_=ind_d)
            id_sb = consts.tile([128, 128], bf16)
            nc.sync.dma_start(out=id_sb[:], in_=id_d)

            gcols = consts.tile([128, 6], f32)   # g00 g01 g10 g11 b0 b1
            for k in range(4):
                nc.gpsimd.dma_start(
                    out=gcols[:, k:k + 1],
                    in_=bass.AP(tensor=gamma_d.tensor, offset=k,
                                ap=[[0, 128], [1, 1]]))
            for k in range(2):
                nc.gpsimd.dma_start(
                    out=gcols[:, 4 + k:5 + k],
                    in_=bass.AP(tensor=beta_d.tensor, offset=k,
                                ap=[[0, 128], [1, 1]]))
            g00c, g01c = gcols[:, 0:1], gcols[:, 1:2]
            g10c, g11c = gcols[:, 2:3], gcols[:, 3:4]
            b0c, b1c = gcols[:, 4:5], gcols[:, 5:6]

            # resident bf16 copy of z for the whole core (64 KiB/partition)
            zb_all = zres.tile([128, NT * COLS], bf16)

            def _pipeline():
                # ---------- phase A: moments into PSUM partition strips ----
                with tc.tile_pool(name="psA", bufs=1, space="PSUM") as psA:
                    # S (cols 0:1024) | Q (1024:2048) | X (2048:2560)
                    ps_stats = psA.tile([128, 2 * COLS + FP], f32)
                    for t in range(NT):
                        g, i = divmod(t, 8)
                        stag = stagp.tile([128, COLS], f32, tag="stag")
                        # alternate the two HWDGE rings (SP / ACT)
                        dma_eng = nc.sync if t % 2 == 0 else nc.scalar
                        dma_eng.dma_start(out=stag[:], in_=z_r[t])
                        if True:
                            zf = stag[:]
                            zb = zb_all[:, t * COLS:(t + 1) * COLS]
                            nc.scalar.copy(zb, zf)          # fp32 -> bf16
                            zz = work.tile([128, COLS], bf16, tag="zz")
                            nc.vector.tensor_tensor(zz[:], zb, zb, OP.mult)
                            zb_ev = bass.AP(tensor=zb_all.tensor,
                                            offset=t * COLS,
                                            ap=[list(zb_all.ap[0]), [2, FP]])
                            zb_od = bass.AP(tensor=zb_all.tensor,
                                            offset=t * COLS + 1,
                                            ap=[list(zb_all.ap[0]), [2, FP]])
                            zx = work.tile([128, FP], bf16, tag="zx")
                            nc.gpsimd.tensor_tensor(zx[:], zb_ev, zb_od,
                                                    OP.mult)

                            lhs = sel_sb[:, 32 * i:32 * (i + 1)]
                            st, sp = (i == 0), (i == 7)
                            rows = slice(32 * g, 32 * (g + 1))
                            tp = (0, 32 * g)
                            for h in range(2):
                                cs = slice(h * NB, (h + 1) * NB)
                                qs = slice(COLS + h * NB, COLS + (h + 1) * NB)
                                nc.tensor.matmul(ps_stats[rows, cs], lhs,
                                                 zb[:, cs], start=st, stop=sp,
                                                 tile_position=tp,
                                                 skip_group_check=True)
                                nc.tensor.matmul(ps_stats[rows, qs], lhs,
                                                 zz[:, cs], start=st, stop=sp,
                                                 tile_position=tp,
                                                 skip_group_check=True)
                            nc.tensor.matmul(ps_stats[rows, 2 * COLS:], lhs,
                                             zx[:], start=st, stop=sp,
                                             tile_position=tp,
                                             skip_group_check=True)

                    # ------ phase B: closed-form 2x2 inverse sqrt + gamma ---
                    ps_S = ps_stats[:, 0:COLS]
                    ps_Q = ps_stats[:, COLS:2 * COLS]
                    ps_X = ps_stats[:, 2 * COLS:]

                    def ev(t_, n=FP):
                        return bass.AP(tensor=t_.tensor, offset=t_.offset,
                                       ap=[list(t_.ap[0]), [2, n]])

                    def od(t_, n=FP):
                        return bass.AP(tensor=t_.tensor, offset=t_.offset + 1,
                                       ap=[list(t_.ap[0]), [2, n]])

                    def dup(t_, n=FP):    # [n] -> [n,2] with stride-0 inner
                        return bass.AP(tensor=t_.tensor, offset=t_.offset,
                                       ap=[list(t_.ap[0]), [1, n], [0, 2]])

                    def pair(t_, n=FP):   # [2n] viewed as [n,2]
                        return bass.AP(tensor=t_.tensor, offset=t_.offset,
                                       ap=[list(t_.ap[0]), [2, n], [1, 2]])

                    Sint = ph2.tile([128, COLS], f32)
                    Pint = ph2.tile([128, COLS], f32)
                    Qint = ph2.tile([128, COLS], f32)
                    Xp = ph2.tile([128, FP], f32)
                    P01 = ph2.tile([128, FP], f32)
                    d1 = ph2.tile([128, FP], f32)
                    s_ = ph2.tile([128, FP], f32)
                    u_ = ph2.tile([128, FP], f32)
                    r_ = ph2.tile([128, FP], f32)
                    # aliases onto dead scratch (lifetimes are disjoint):
                    q2 = P01     # P01 dead once sigma01 subtract is done
                    u2 = d1      # d1 dead once s_ = sqrt(d1)
                    tq = u_      # u_ dead once u2 = 2s + u_
                    rsc = P01    # q2 dead once det -= q2
                    T_ = s_      # s_ dead once e_int += s dup
                    c0 = d1      # u2 dead once tq = sqrt((B-1) u2)
                    c1 = r_     # r_ dead once w01 *= r
                    Pap = ph2.tile([128, COLS], bf16)
                    Qap = ph2.tile([128, COLS], bf16)
                    Rap = ph2.tile([128, COLS], bf16)

                    # evacuate S; Pint = (S/sqrt(B))^2 per component
                    nc.scalar.copy(Sint[:], ps_S)
                    nc.scalar.activation(Pint[:], ps_S, AF.Square,
                                         scale=float(irB))
                    # P01 = (S0/B)*S1
                    nc.vector.scalar_tensor_tensor(P01, ev(Sint),
                                                   float(irB * irB),
                                                   od(Sint), OP.mult, OP.mult)
                    # sigma~ = Q - P (evacuates Q), X - P01 (evacuates X)
                    nc.vector.tensor_tensor(Qint[:], ps_Q, Pint, OP.subtract)
                    nc.vector.tensor_tensor(Xp[:], ps_X, P01, OP.subtract)
                    # det
                    nc.gpsimd.tensor_tensor(d1, ev(Qint), od(Qint), OP.mult)
                    nc.scalar.square(q2, Xp)
                    nc.vector.tensor_tensor(d1, d1, q2, OP.subtract)
                    nc.scalar.activation(s_, d1, AF.Sqrt)
                    # u = trace~ + 2 s~
                    nc.gpsimd.tensor_tensor(u_, ev(Qint), od(Qint), OP.add)
                    nc.vector.scalar_tensor_tensor(u2, s_, 2.0, u_, OP.mult,
                                                   OP.add)
                    # tq = sqrt((B-1) u) = (B-1) t ;  r = 1/tq
                    nc.scalar.activation(tq, u2, AF.Sqrt, scale=float(B - 1))
                    nc.vector.reciprocal_approx_accurate(r_, tq, rsc)
                    # e = sigma~ + s~ I (dup) ; W = e*r ; w01 = sigma01 * r
                    nc.vector.tensor_tensor(pair(Qint), pair(Qint), dup(s_),
                                            OP.add)
                    nc.vector.tensor_tensor(pair(Qint), pair(Qint), dup(r_),
                                            OP.mult)
                    nc.vector.tensor_tensor(Xp[:], Xp[:], r_[:], OP.mult)
                    w00, w11, w01 = ev(Qint), od(Qint), Xp

                    # A coefficients -> interleaved bf16 apply planes
                    # Pap = (A00, A10): A00 = g00 w00 + g01 w01
                    nc.vector.tensor_scalar(T_, w00, g00c, None, OP.mult)
                    nc.vector.scalar_tensor_tensor(ev(Pap), w01, g01c, T_,
                                                   OP.mult, OP.add)
                    nc.vector.tensor_scalar(T_, w00, g10c, None, OP.mult)
                    nc.vector.scalar_tensor_tensor(od(Pap), w01, g11c, T_,
                                                   OP.mult, OP.add)
                    # Qap = (A01, A11): A01 = g00 w01 + g01 w11
                    nc.vector.tensor_scalar(T_, w11, g01c, None, OP.mult)
                    nc.vector.scalar_tensor_tensor(ev(Qap), w01, g00c, T_,
                                                   OP.mult, OP.add)
                    nc.vector.tensor_scalar(T_, w11, g11c, None, OP.mult)
                    nc.vector.scalar_tensor_tensor(od(Qap), w01, g10c, T_,
                                                   OP.mult, OP.add)
                    # Rap = (R0, R1): R0 = b0 - (A00 S0 + A01 S1)/B
                    nc.gpsimd.tensor_tensor(c0, ev(Pap), ev(Sint), OP.mult)
                    nc.gpsimd.tensor_tensor(c1, ev(Qap), od(Sint), OP.mult)
                    nc.gpsimd.tensor_tensor(c0, c0, c1, OP.add)
                    nc.vector.tensor_scalar(ev(Rap), c0, float(-1.0 / B),
                                            b0c, OP.mult, OP.add)
                    nc.gpsimd.tensor_tensor(c0, od(Pap), ev(Sint), OP.mult)
                    nc.gpsimd.tensor_tensor(c1, od(Qap), od(Sint), OP.mult)
                    nc.gpsimd.tensor_tensor(c0, c0, c1, OP.add)
                    nc.vector.tensor_scalar(od(Rap), c0, float(-1.0 / B),
                                            b1c, OP.mult, OP.add)

                # ---------- phase C: broadcast + apply ----------
                # out = P.z_ev + Q.z_od + R assembled entirely in PSUM: the
                # indicator matmuls broadcast R (start of an accumulation
                # group), DVE writes the two products t1/t2 as bf16, and two
                # identity matmuls accumulate them onto R in PSUM.  ScalarE
                # evacuates the finished plane.  GPSIMD does nothing here;
                # DVE only the two products.  PSUM: P 2 + Q 2 + R 2x2 = 8.
                with (
                    tc.tile_pool(name="ps3p", bufs=1, space="PSUM") as ps3p,
                    tc.tile_pool(name="ps3q", bufs=1, space="PSUM") as ps3q,
                    tc.tile_pool(name="ps3r", bufs=2, space="PSUM") as ps3r,
                ):
                    for t in range(NT):
                        g, i = divmod(t, 8)
                        rows = slice(32 * g, 32 * (g + 1))
                        lhs_b = ind_sb[rows, 128 * i:128 * (i + 1)]
                        bP = ps3p.tile([128, COLS], f32, tag="psP")
                        bQ = ps3q.tile([128, COLS], f32, tag="psQ")
                        bR = ps3r.tile([128, COLS], f32, tag="psR")
                        for h in range(2):
                            cs = slice(h * NB, (h + 1) * NB)
                            nc.tensor.matmul(bP[:, cs], lhs_b, Pap[rows, cs],
                                             start=True, stop=True,
                                             tile_position=(32 * g, 0),
                                             skip_group_check=True)
                            nc.tensor.matmul(bQ[:, cs], lhs_b, Qap[rows, cs],
                                             start=True, stop=True,
                                             tile_position=(32 * g, 0),
                                             skip_group_check=True)
                            nc.tensor.matmul(bR[:, cs], lhs_b,
                                             Rap[rows, cs],
                                             start=True, stop=False,
                                             tile_position=(32 * g, 0),
                                             skip_group_check=True)
                        zoff = t * COLS
                        zdup_ev = bass.AP(tensor=zb_all.tensor, offset=zoff,
                                          ap=[list(zb_all.ap[0]), [2, FP],
                                              [0, 2]])
                        zdup_od = bass.AP(tensor=zb_all.tensor,
                                          offset=zoff + 1,
                                          ap=[list(zb_all.ap[0]), [2, FP],
                                              [0, 2]])
                        t1 = work.tile([128, COLS], bf16, tag="t1")
                        t2 = work.tile([128, COLS], bf16, tag="t2")
                        nc.vector.tensor_tensor(pair(t1), pair(bP), zdup_ev,
                                                OP.mult)
                        nc.vector.tensor_tensor(pair(t2), pair(bQ), zdup_od,
                                                OP.mult)
                        for h in range(2):
                            cs = slice(h * NB, (h + 1) * NB)
                            nc.tensor.matmul(bR[:, cs], id_sb[:],
                                             t1[:, cs],
                                             start=False, stop=False,
                                             tile_position=(0, 0),
                                             skip_group_check=True)
                            nc.tensor.matmul(bR[:, cs], id_sb[:],
                                             t2[:, cs],
                                             start=False, stop=True,
                                             tile_position=(0, 0),
                                             skip_group_check=True)
                        obuf = work.tile([128, COLS], f32, tag="outb")
                        nc.scalar.copy(obuf[:], bR[:])
                        dma_eng = nc.scalar if t % 2 == 0 else nc.sync
                        dma_eng.dma_start(out=out_r[t], in_=obuf[:])

            for _rep in range(reps):
                _pipeline()

    nc.compile()
    return nc


_NC = {}


def _get_module(reps=1):
    if reps not in _NC:
        _NC[reps] = build_module(reps)
    return _NC[reps]


def kernel(z, gamma, beta):
    z = np.ascontiguousarray(z, dtype=np.float32)
    gamma = np.ascontiguousarray(gamma, dtype=np.float32)
    beta = np.ascontiguousarray(beta, dtype=np.float32)
    zr = z.reshape(B, C, H * W * 2)
    sel8, ind, id128 = _host_constants()
    in_maps = []
    for c in range(NCORES):
        shard = np.ascontiguousarray(
            zr[:, c * C_PER:(c + 1) * C_PER].reshape(B, M))
        in_maps.append({"z": shard, "gamma": gamma, "beta": beta,
                        "sel8": sel8, "ind": ind, "id128": id128})
    m1 = _get_module(1)
    runner = _get_runner(("m", id(m1)), m1, NCORES)
    results = _run_module(runner, in_maps)
    out = np.empty((B, C, H * W * 2), dtype=np.float32)
    for c in range(NCORES):
        out[:, c * C_PER:(c + 1) * C_PER] = results[c]["out"].reshape(
            B, C_PER, H * W * 2)
    return out.reshape(B, C, H, W, 2)


def _build_memcpy_module(reps=1):
    """Baseline: per-core DMA z -> out through SBUF (same traffic as kernel)."""
    nc = bacc.Bacc("TRN2", target_bir_lowering=False, debug=False,
                   detect_race_conditions=False)
    z_d = nc.dram_tensor("z", [B, M], f32, kind="ExternalInput").ap()
    out_d = nc.dram_tensor("out", [B, M], f32, kind="ExternalOutput").ap()
    z_r = z_d.rearrange("b (t j f) -> t j b f", t=NT, j=J, f=COLS)
    out_r = out_d.rearrange("b (t j f) -> t j b f", t=NT, j=J, f=COLS)
    with tile.TileContext(nc) as tc:
        with tc.tile_pool(name="buf", bufs=6) as buf:
            for _ in range(reps):
                for t in range(NT):
                    x = buf.tile([128, COLS], f32, tag="x")
                    nc.sync.dma_start(out=x[:], in_=z_r[t])
                    nc.scalar.dma_start(out=out_r[t], in_=x[:])
    nc.compile()
    return nc


def bench_memcpy(z, iters=10, reps=17):
    z = np.ascontiguousarray(z, dtype=np.float32)
    zr = z.reshape(B, C, H * W * 2)
    in_maps = []
    for c in range(NCORES):
        shard = np.ascontiguousarray(
            zr[:, c * C_PER:(c + 1) * C_PER].reshape(B, M))
        in_maps.append({"z": shard})
    ta, tb = bench_pair((_build_memcpy_module(1), _build_memcpy_module(reps)),
                        in_maps, in_maps, iters=iters, rounds=4)
    slopes = sorted((b - a) / (reps - 1) for a, b in zip(ta, tb))
    return slopes[len(slopes) // 2]


def _make_runner(nc, n_cores):
    """Build (and cache) the sharded jit executable for an SPMD module."""
    import jax
    import jax.numpy as jnp
    from jax.sharding import Mesh, PartitionSpec
    from jax.experimental.shard_map import shard_map
    from concourse import bass2jax
    from concourse.bass2jax import _bass_exec_p, install_neuronx_cc_hook
    from concourse import mybir as _mb

    install_neuronx_cc_hook()
    partition_name = (nc.partition_id_tensor.name
                      if nc.partition_id_tensor else None)
    in_names, out_names, out_avals, zero_outs = [], [], [], []
    for alloc in nc.m.functions[0].allocations:
        if not isinstance(alloc, _mb.MemoryLocationSet):
            continue
        name = alloc.memorylocations[0].name
        if alloc.kind == "ExternalInput":
            if name != partition_name:
                in_names.append(name)
        elif alloc.kind == "ExternalOutput":
            shape = tuple(alloc.tensor_shape)
            dtype = _mb.dt.np(alloc.dtype)
            out_names.append(name)
            out_avals.append(jax.core.ShapedArray(shape, dtype))
            zero_outs.append(np.zeros(shape, dtype))
    n_params = len(in_names)
    n_outs = len(out_avals)
    all_in_names = in_names + out_names
    if partition_name is not None:
        all_in_names.append(partition_name)

    def _body(*args):
        operands = list(args)
        if partition_name is not None:
            operands.append(bass2jax.partition_id_tensor())
        outs = _bass_exec_p.bind(
            *operands,
            out_avals=tuple(out_avals),
            in_names=tuple(all_in_names),
            out_names=tuple(out_names),
            lowering_input_output_aliases=(),
            sim_require_finite=True,
            sim_require_nnan=True,
            nc=nc,
        )
        return tuple(outs)

    devices = jax.devices()[:n_cores]
    mesh = Mesh(np.asarray(devices), ("core",))
    donate = tuple(range(n_params, n_params + n_outs))
    sharded = jax.jit(
        shard_map(_body, mesh=mesh,
                  in_specs=(PartitionSpec("core"),) * (n_params + n_outs),
                  out_specs=(PartitionSpec("core"),) * n_outs,
                  check_rep=False),
        donate_argnums=donate, keep_unused=True,
    )
    from jax.sharding import NamedSharding
    shard0 = NamedSharding(mesh, PartitionSpec("core"))
    return {
        "sharded": sharded, "shard0": shard0, "in_names": in_names,
        "out_names": out_names, "out_avals": out_avals,
        "zero_outs": zero_outs, "n_cores": n_cores,
    }


_RUNNERS = {}


def _get_runner(key, nc, n_cores):
    if key not in _RUNNERS:
        _RUNNERS[key] = _make_runner(nc, n_cores)
    return _RUNNERS[key]


def _run_module(runner, in_maps):
    import jax
    n_cores = runner["n_cores"]
    concat_in = [
        jax.device_put(
            np.concatenate([np.asarray(m[name]) for m in in_maps], axis=0),
            runner["shard0"])
        for name in runner["in_names"]
    ]
    zeros = [
        jax.device_put(
            np.zeros((n_cores * z.shape[0], *z.shape[1:]), z.dtype),
            runner["shard0"])
        for z in runner["zero_outs"]
    ]
    outs = runner["sharded"](*concat_in, *zeros)
    jax.block_until_ready(outs)
    return [
        {name: np.asarray(outs[i]).reshape(
            n_cores, *runner["out_avals"][i].shape)[c]
         for i, name in enumerate(runner["out_names"])}
        for c in range(n_cores)
    ]


def bench_module(nc, in_maps, iters=12, key=None):
    """Min-of-per-call timing of an SPMD bass module via the PJRT path."""
    import time as _time
    import jax
    runner = _make_runner(nc, len(in_maps))
    n_cores = runner["n_cores"]
    shard0 = runner["shard0"]
    sharded = runner["sharded"]
    concat_in = [
        jax.device_put(
            np.concatenate([np.asarray(m[name]) for m in in_maps], axis=0),
            shard0)
        for name in runner["in_names"]
    ]
    zero_sets = []
    for _ in range(iters + 1):
        zero_sets.append([
            jax.device_put(
                np.zeros((n_cores * z.shape[0], *z.shape[1:]), z.dtype),
                shard0)
            for z in runner["zero_outs"]
        ])
    outs = sharded(*concat_in, *zero_sets[0])
    jax.block_until_ready(outs)

    def one_batch(ks):
        t0 = _time.perf_counter()
        last = None
        for k in ks:
            last = sharded(*concat_in, *zero_sets[k + 1])
        jax.block_until_ready(last)
        return (_time.perf_counter() - t0) / len(ks), last

    dt, last = one_batch(range(iters))
    results = [
        {name: np.asarray(last[i]).reshape(
            n_cores, *runner["out_avals"][i].shape)[c]
         for i, name in enumerate(runner["out_names"])}
        for c in range(n_cores)
    ]
    return dt * 1e9, results


def bench_pair(ncs, in_maps_a, in_maps_b, iters=8, rounds=4):
    """Interleaved async-batch timing of two modules; returns
    (median per-call ns A, median per-call ns B, per-round lists)."""
    import time as _time
    import jax
    runners = [_get_runner(("m", id(ncs[0])), ncs[0], len(in_maps_a)),
               _get_runner(("m", id(ncs[1])), ncs[1], len(in_maps_b))]
    sides = []
    for runner, im in ((runners[0], in_maps_a), (runners[1], in_maps_b)):
        concat_in = [
            jax.device_put(
                np.concatenate([np.asarray(m[name]) for m in im], axis=0),
                runner["shard0"])
            for name in runner["in_names"]
        ]
        n_cores = runner["n_cores"]
        zsets = []
        for _ in range(iters * rounds + 1):
            zsets.append([
                jax.device_put(
                    np.zeros((n_cores * z.shape[0], *z.shape[1:]), z.dtype),
                    runner["shard0"])
                for z in runner["zero_outs"]
            ])
        sides.append((runner, concat_in, zsets))
        out = runner["sharded"](*concat_in, *zsets[0])
        jax.block_until_ready(out)
    ta, tb = [], []
    k = [0, 0]
    for r in range(rounds):
        for side, rec in ((0, ta), (1, tb)):
            runner, concat_in, zsets = sides[side]
            t0 = _time.perf_counter()
            last = None
            for _ in range(iters):
                k[side] += 1
                last = runner["sharded"](*concat_in, *zsets[k[side]])
            jax.block_until_ready(last)
            rec.append((_time.perf_counter() - t0) / iters * 1e9)
    return ta, tb


def bench(z, gamma, beta, iters=10, reps=17, with_memcpy=False):
    """Slope-based device timing: time modules with `reps`=1 and `reps`=R
    internal repetitions of the full pipeline; per-kernel device time =
    (t_R - t_1) / (R - 1), which cancels the per-dispatch axon overhead."""
    z = np.ascontiguousarray(z, dtype=np.float32)
    zr = z.reshape(B, C, H * W * 2)
    sel8, ind, id128 = _host_constants()
    in_maps = []
    for c in range(NCORES):
        shard = np.ascontiguousarray(
            zr[:, c * C_PER:(c + 1) * C_PER].reshape(B, M))
        in_maps.append({"z": shard,
                        "gamma": np.ascontiguousarray(gamma, np.float32),
                        "beta": np.ascontiguousarray(beta, np.float32),
                        "sel8": sel8, "ind": ind, "id128": id128})
    ta, tb = bench_pair((_get_module(1), _get_module(reps)),
                        in_maps, in_maps, iters=iters, rounds=8)
    slopes = sorted((b - a) / (reps - 1) for a, b in zip(ta, tb))
    ns = slopes[len(slopes) // 2]
    m1 = _get_module(1)
    runner = _get_runner(("m", id(m1)), m1, NCORES)
    results = _run_module(runner, in_maps)
    t1_ns, tR_ns = min(ta), min(tb)
    out = np.empty((B, C, H * W * 2), dtype=np.float32)
    for c in range(NCORES):
        out[:, c * C_PER:(c + 1) * C_PER] = results[c]["out"].reshape(
            B, C_PER, H * W * 2)
    return out.reshape(B, C, H, W, 2), ns, (t1_ns, tR_ns)


def run_traced(z, gamma, beta):
    """Like kernel() but with NTFF tracing; returns (output, exec_time_ns)."""
    z = np.ascontiguousarray(z, dtype=np.float32)
    zr = z.reshape(B, C, H * W * 2)
    sel8, ind, id128 = _host_constants()
    in_maps = []
    for c in range(NCORES):
        shard = np.ascontiguousarray(
            zr[:, c * C_PER:(c + 1) * C_PER].reshape(B, M))
        in_maps.append({"z": shard,
                        "gamma": np.ascontiguousarray(gamma, np.float32),
                        "beta": np.ascontiguousarray(beta, np.float32),
                        "sel8": sel8, "ind": ind, "id128": id128})
    nc = _get_module()
    res = run_bass_kernel_spmd(nc, in_maps, core_ids=list(range(NCORES)),
                               trace=True)
    out = np.empty((B, C, H * W * 2), dtype=np.float32)
    for c in range(NCORES):
        out[:, c * C_PER:(c + 1) * C_PER] = res.results[c]["out"].reshape(
            B, C_PER, H * W * 2)
    return out.reshape(B, C, H, W, 2), res.exec_time_ns, res

